# revision 54
# baseline (speedup 1.0000x reference)
"""Binarized linear layer (BLinear) Trainium2 kernel, v6 (74.4us/iter HW).

Computes y = sign(x) @ sign(W).T + b for x [8192, 2048] f32, W [2048, 2048] f32,
b [2048] f32. Data-parallel across 8 NeuronCores (1024 tokens per core, W
replicated).

Math notes (all exact => bit-exact vs the fp32 reference):
 - sign() in {-1, 0, +1} is exact in bf16/fp8e4; TensorE accumulates fp32 in
   PSUM; sums of +-1 over K=2048 are exact integers << 2^24.
 - x and W are staged to DRAM as bf16 (host cast; bf16 keeps fp32's exponent
   range so sign()/zeroness is preserved for every fp32 input), host-permuted
   to the contraction-major layout [chunk][ki, ko, t|o] so every device load
   is a plain fully-contiguous-per-partition DMA (no xbar DMA-transpose).
 - y is computed TRANSPOSED on device (yT [2048 o, 1024 t] fp16; integer sums
   with |y| <= 2048 are exact in fp16; bias added on-device from f32 PSUM
   before the cast) and un-transposed/widened on the host.

Structure (96.4us v1 -> 74.4us v6; mm-only floor measured 71.5us):
 - fp8e4 DoubleRow matmuls, W stationary: lhsT = wb [128ki, 2ko, 128o] reused
   by 2 moving-x matmuls (rhs = xb [128ki, 2ko, 512t], out psum [128t?o, 512]).
   Probes showed LDWEIGHTS switch cost and ifmap interleaving are ~free; the
   ~280ns/MM (vs 241 ideal) is intrinsic (P0 clock under sustained load).
 - The timing loop body holds EIGHT unrolled copies with explicit
   tc.stage_boundary() every 2 copies (staggered_reset=True): stage ==
   2 iterations, so the adjacent-stage rule gives copy N+1's prep a full
   matmul-span of lookahead. (The auto equal split cuts mid-MM-stream and
   couples PE to the DMA stream; full-barrier mode kills cross-body overlap.)
 - ENGINE-PHASE PURITY (the critical fix): every engine's FIFO queue holds
   only same-phase work, so copy N's late-paced work never blocks copy N+1's
   prep: SP = loads only; ACT = early signs only (x0,x1,w0-w4, native Sign);
   DVE = late signs (w5-w7, two-op min/max form, exact for |v| >= 2^-126)
   then PSUM evictions (tensor_scalar_add with per-partition bias AP);
   GPS/SWDGE = store + bias DMA issues only (GPS tensor compute is ~45us/op —
   never use it for elementwise).
 - All operand pools bufs=2 (copies alternate slots) => cross-iteration
   software pipelining; staging pools sized so WAR slot releases land early.
"""

import numpy as np

N_CORES = 8
TOKENS = 8192
D_IN = 2048
D_OUT = 2048
T_CORE = TOKENS // N_CORES  # 1024 tokens per core

P = 128
KO = D_IN // P          # 16 contraction chunks of 128
KP = KO // 2            # 8 DoubleRow K-pairs (256 per matmul)
NB = 512                # matmul moving free dim / PSUM bank (fp32)
TH = T_CORE // NB       # 2 token halves of 512
OC = D_OUT // P         # 16 out-feature tiles of 128
WCH = 8                 # W prep chunks (256 o-rows each)
WCO = D_OUT // WCH      # 256 o per W chunk

_CACHE = {}
LAST_RESULT = None

# "wstat": DoubleRow, W stationary, wb [P, KO, WCO]
# "wswi":  DoubleRowSwInterleave — host pre-interleaves W so LDWEIGHTS reads
#          contiguously; wb [P, KP, 2, 2P]
MM_STRUCT = "wstat"


def _build_bass(loop_n=1, phase="all", mm_struct=None, sign_gps=False,
                loop_mode="staggered8"):
    if mm_struct is None:
        mm_struct = MM_STRUCT
    import concourse.mybir as mybir
    import concourse.tile as tile
    from concourse import bacc
    from concourse.bass import ts

    nc = bacc.Bacc(
        "TRN2",
        target_bir_lowering=False,
        debug=False,
        enable_asserts=False,
    )

    f32 = mybir.dt.float32
    bf16 = mybir.dt.bfloat16
    fp16 = mybir.dt.float16
    fp8 = mybir.dt.float8e4

    # xp/Wp are host-permuted to the contraction-major SBUF layout
    # [chunk][ki, ko, t|o] (bit-exact bf16 values, pure layout staging), so
    # each device load is one fully-contiguous-per-partition plain DMA — no
    # xbar DMA-transpose, max descriptor efficiency.
    x_d = nc.dram_tensor("xp", [TH, P, KO, NB], bf16, kind="ExternalInput")
    if mm_struct == "wswi":
        w_d = nc.dram_tensor("Wp", [WCH, P, KP, 2, 2 * P], bf16,
                             kind="ExternalInput")
    else:
        w_d = nc.dram_tensor("Wp", [WCH, P, KO, WCO], bf16, kind="ExternalInput")
    b_d = nc.dram_tensor("bt", [P, OC], f32, kind="ExternalInput")
    y_d = nc.dram_tensor("yT", [D_OUT, T_CORE], fp16, kind="ExternalOutput")

    x_ap = x_d.ap()
    w_ap = w_d.ap()
    b_ap = b_d.ap()
    y_ap = y_d.ap()

    if loop_n <= 1:
        unroll = 1
    elif loop_mode == "staggered8":
        unroll = 8
    else:
        unroll = 4

    with tile.TileContext(nc) as tc:
        with (
            tc.tile_pool(name="ops", bufs=2) as ops,
            tc.tile_pool(name="xstage", bufs=2) as xstage,
            tc.tile_pool(name="wstage", bufs=6) as wstage,
            tc.tile_pool(name="dvetmp", bufs=1) as dvetmp,
            tc.tile_pool(name="outp", bufs=3) as out_pool,
            tc.tile_pool(name="psum", bufs=8, space="PSUM") as psum_pool,
        ):
            ev_i = 0

            def body_one(u):
                nonlocal ev_i
                # --- operand tiles for this (unrolled) iteration ---
                if mm_struct == "xswi":
                    # moving-operand pairs contiguous: [ki, kp, t, b]
                    xb = [ops.tile([P, KP, NB, 2], fp8, name=f"xb{h}")
                          for h in range(TH)]
                else:
                    xb = [ops.tile([P, KO, NB], fp8, name=f"xb{h}") for h in range(TH)]
                if mm_struct in ("wstat", "wsame", "xswi"):
                    wb = [ops.tile([P, KO, WCO], fp8, name=f"wb{c}") for c in range(WCH)]
                elif mm_struct == "wswi":
                    # SwInterleave storage: per (kp, lo) a contiguous 256-elem
                    # block [A_col127, B_col127, A_col126, ..., B_col0]
                    wb = [ops.tile([P, KP, 2, 2 * P], fp8, name=f"wb{c}")
                          for c in range(WCH)]
                else:
                    wb = [ops.tile([P, KO, NB], fp8, name=f"wb{c}") for c in range(4)]
                bias = ops.tile([P, OC], f32, name="bias")

                if phase == "mm":
                    # timing-only build: tiny slice writes allocate the tiles
                    # (full contents are garbage; numerics unused)
                    for t_ in xb + wb:
                        if (mm_struct == "wswi" and t_ in wb) or (
                            mm_struct == "xswi" and t_ in xb
                        ):
                            nc.gpsimd.memset(t_[:, 0, 0, 0:1], 1.0)
                        else:
                            nc.gpsimd.memset(t_[:, 0, 0:1], 1.0)
                    nc.gpsimd.memset(bias[:, 0:1], 0.0)
                else:
                    # --- prep: DMA-transpose from DRAM (bf16) + sign -> fp8 ---
                    nc.gpsimd.dma_start(bias[:], b_ap[:, :])

                    def sign_act(dst, src):
                        nc.scalar.sign(dst, src)

                    def sign_2op(eng, dst, src, shape):
                        # exact sign for all |v| >= 2^-126 (incl. v == 0)
                        tmp = dvetmp.tile(shape, bf16, name="dvetmp")
                        eng.tensor_scalar(
                            tmp[:], src, 2.0 ** 126, 1.0,
                            mybir.AluOpType.mult, mybir.AluOpType.min,
                        )
                        eng.tensor_scalar_max(dst, tmp[:], -1.0)

                    def prep_x(h):
                        st = xstage.tile([P, KO, NB], bf16, name="xst")
                        nc.sync.dma_start(st[:], x_ap[h])
                        if phase == "dma":
                            nc.vector.tensor_copy(xb[h][:, 0, 0:1], st[:, 0, 0:1])
                            return
                        sign_act(xb[h][:], st[:])

                    def prep_w(c):
                        wst_shape = (
                            [P, KP, 2, 2 * P] if mm_struct == "wswi"
                            else [P, KO, WCO]
                        )
                        st = wstage.tile(wst_shape, bf16, name="wst")
                        nc.sync.dma_start(st[:], w_ap[c])
                        if phase == "dma":
                            dst = (wb[c][:, 0, 0, 0:1] if mm_struct == "wswi"
                                   else wb[c][:, 0, 0:1])
                            nc.vector.tensor_copy(dst, st[:, 0, 0:1]
                                                  if mm_struct != "wswi"
                                                  else st[:, 0, 0, 0:1])
                            return
                        # DVE gets the LATE-consumed chunks (w5..w7): its queue
                        # drains copy u's evictions first, so u+1's DVE signs
                        # land mid-mm — early chunks must come from ACT, whose
                        # queue holds only signs and drains well before u ends.
                        if c >= 5:
                            sign_2op(nc.vector, wb[c][:], st[:], wst_shape)
                        else:
                            sign_act(wb[c][:], st[:])

                    prep_x(0)
                    prep_w(0)
                    prep_x(1)
                    for c in range(1, WCH):
                        prep_w(c)

                if phase in ("prep", "dma"):
                    # tiny consumers so prep work can't be dead-code'd away
                    o_sb = out_pool.tile([P, NB], fp16, tag="osb", name="o_sb")
                    for i, t_ in enumerate(xb + wb):
                        src = (t_[:, 0, 0, 0:1]
                               if mm_struct == "wswi" and t_ not in xb
                               else t_[:, 0, 0:1])
                        nc.vector.tensor_copy(o_sb[:, i : i + 1], src)
                    nc.scalar.dma_start(y_ap[ts(0, P), ts(0, NB)], o_sb[:])
                    return

                if mm_struct in ("wstat", "wsame", "wswi", "xswi"):
                    # th INNER: each stationary W tile (LDWEIGHTS) is reused by
                    # the 2 moving-x matmuls — measured ~5us/iter cheaper than
                    # reloading per MM (th-outer). In steady state prep(u+1)
                    # completes during mm(u), so x-half gating doesn't matter.
                    for oc in range(OC):
                        c, lo = divmod(oc, 2)
                        if mm_struct == "wsame":
                            c, lo = 0, 0  # fixed stationary: LDW-elision probe
                        psums = [
                            psum_pool.tile([P, NB], f32, tag="psum", name="psum")
                            for _ in range(TH)
                        ]
                        for kp in range(KP):
                            kp_ = 0 if mm_struct == "wsame" else kp
                            if mm_struct == "wswi":
                                lhsT = wb[c][:, kp_, lo, :]
                                pm = mybir.MatmulPerfMode.DoubleRowSwInterleave
                            else:
                                lhsT = wb[c][:, 2 * kp_ : 2 * kp_ + 2, ts(lo, P)]
                                pm = mybir.MatmulPerfMode.DoubleRow
                            for th in range(TH):
                                if mm_struct == "xswi":
                                    rhs = xb[th][:, kp, :, :].transpose([0, 2, 1])
                                else:
                                    rhs = xb[th][:, 2 * kp : 2 * kp + 2, :]
                                nc.tensor.matmul(
                                    psums[th][:],
                                    lhsT=lhsT,
                                    rhs=rhs,
                                    perf_mode=pm,
                                    start=(kp == 0),
                                    stop=(kp == KP - 1),
                                )
                        for th in range(TH):
                            o_sb = out_pool.tile([P, NB], fp16, tag="osb", name="o_sb")
                            # evictions on DVE (ACT Identity would force
                            # activation-table switches against the Sign ops)
                            nc.vector.tensor_scalar_add(
                                o_sb[:], psums[th][:], bias[:, oc : oc + 1]
                            )
                            ev_i += 1
                            # stores issue via GPS/SWDGE: the store issues are
                            # paced by the mm span, and any engine that also
                            # did sign work would FIFO-block the next copy's
                            # signs behind them. GPS does only stores + bias.
                            nc.gpsimd.dma_start(y_ap[ts(oc, P), ts(th, NB)], o_sb[:])
                else:
                    # xstat: v1-style — stationary x token-tile, moving W bank
                    # [128, 2, 512]; timing-only build (phase="mm").
                    assert phase == "mm"
                    for ob in range(4):
                        for tt in range(8):
                            th, tl = divmod(tt, 4)
                            psum = psum_pool.tile([P, NB], f32, tag="psum", name="psum")
                            for kp in range(KP):
                                nc.tensor.matmul(
                                    psum[:],
                                    lhsT=xb[th][:, 2 * kp : 2 * kp + 2, ts(tl, P)],
                                    rhs=wb[ob][:, 2 * kp : 2 * kp + 2, :],
                                    perf_mode=mybir.MatmulPerfMode.DoubleRow,
                                    start=(kp == 0),
                                    stop=(kp == KP - 1),
                                )
                            o_sb = out_pool.tile([P, NB], fp16, tag="osb", name="o_sb")
                            nc.vector.tensor_scalar_add(
                                o_sb[:], psum[:], bias[:, 0:1]
                            )
                            # timing-only: yT is [2048, 1024]; write any
                            # distinct in-range region per (ob, tt)
                            nc.scalar.dma_start(
                                y_ap[ts(2 * ob + (tt % 2), P), ts(tt // 4, NB)],
                                o_sb[:],
                            )

            staggered = loop_mode in ("staggered4", "staggered8")
            copies_per_stage = unroll // 4 if staggered else unroll

            def body():
                for u in range(unroll):
                    if staggered and u > 0 and u % copies_per_stage == 0:
                        # per-copy-group stage boundaries: stage == 1-2 logical
                        # iterations, so the adjacent-stage rule allows 1-3
                        # copies of prep lookahead (the auto equal split cuts
                        # mid-MM-stream and couples PE to the DMA/sign stream)
                        tc.stage_boundary()
                    body_one(u)

            if loop_n > 1:
                assert loop_n % unroll == 0
                with tc.For_i(
                    0,
                    loop_n // unroll,
                    1,
                    hint_engines=(mybir.EngineType.PE,),
                    staggered_reset=staggered,
                ):
                    body()
            else:
                body()

    nc.compile()
    return nc


def _get_nc():
    if "nc" not in _CACHE:
        _CACHE["nc"] = _build_bass()
    return _CACHE["nc"]


def _host_inputs(inputs):
    import ml_dtypes

    x = np.asarray(inputs["x"], dtype=np.float32)
    W = np.asarray(inputs["W"], dtype=np.float32)
    b = np.ascontiguousarray(np.asarray(inputs["b"], dtype=np.float32))

    # bf16 staging: sign-preserving (bf16 keeps fp32's exponent range);
    # layouts permuted to contraction-major [ki, ko, t|o]
    x16 = x.astype(ml_dtypes.bfloat16)
    W16 = W.astype(ml_dtypes.bfloat16)
    xp = [
        np.ascontiguousarray(
            x16[c * T_CORE : (c + 1) * T_CORE]
            .reshape(TH, NB, KO, P)
            .transpose(0, 3, 2, 1)
        )
        for c in range(N_CORES)
    ]
    if MM_STRUCT == "wswi":
        # SwInterleave weight staging: per (c, ki, kp, lo) a 256-elem block
        # [A_col127, B_col127, A_col126, ..., B_col0]; A/B = ko-even/odd,
        # columns descending (HW deinterleaves then reverses)
        arr = W16.reshape(WCH, 2, P, KP, 2, P)  # [c, lo, m, kp, b, ki]
        Wp = np.ascontiguousarray(
            arr[:, :, ::-1]
            .transpose(0, 5, 3, 1, 2, 4)
            .reshape(WCH, P, KP, 2, 2 * P)
        )
    else:
        Wp = np.ascontiguousarray(
            W16.reshape(WCH, WCO, KO, P).transpose(0, 3, 2, 1)
        )
    # bias transposed to per-partition layout: bt[p, c] = b[c*128 + p]
    bt = np.ascontiguousarray(b.reshape(OC, P).T)
    return xp, Wp, bt


def kernel(**inputs):
    global LAST_RESULT

    from concourse.bass_utils import run_bass_kernel_spmd

    xp, Wp, bt = _host_inputs(inputs)

    nc = _get_nc()
    in_maps = [
        {"xp": xp[c], "Wp": Wp, "bt": bt}
        for c in range(N_CORES)
    ]
    res = run_bass_kernel_spmd(nc, in_maps, core_ids=list(range(N_CORES)))
    LAST_RESULT = res
    # un-transpose per-core yT [2048, 1024] -> y [1024, 2048]; widen to f32
    y = np.concatenate(
        [np.ascontiguousarray(r["yT"].T) for r in res.results], axis=0
    )
    return y.astype(np.float32)


# revision 57
# speedup vs baseline: 1.0341x; 1.0341x over previous
"""Binarized linear layer (BLinear) Trainium2 kernel, v6 (74.4us/iter HW).

Computes y = sign(x) @ sign(W).T + b for x [8192, 2048] f32, W [2048, 2048] f32,
b [2048] f32. Data-parallel across 8 NeuronCores (1024 tokens per core, W
replicated).

Math notes (all exact => bit-exact vs the fp32 reference):
 - sign() in {-1, 0, +1} is exact in bf16/fp8e4; TensorE accumulates fp32 in
   PSUM; sums of +-1 over K=2048 are exact integers << 2^24.
 - x and W are staged to DRAM as bf16 (host cast; bf16 keeps fp32's exponent
   range so sign()/zeroness is preserved for every fp32 input), host-permuted
   to the contraction-major layout [chunk][ki, ko, t|o] so every device load
   is a plain fully-contiguous-per-partition DMA (no xbar DMA-transpose).
 - y is computed TRANSPOSED on device (yT [2048 o, 1024 t] fp16; integer sums
   with |y| <= 2048 are exact in fp16; bias added on-device from f32 PSUM
   before the cast) and un-transposed/widened on the host.

Structure (96.4us v1 -> 74.4us v6; mm-only floor measured 71.5us):
 - fp8e4 DoubleRow matmuls, W stationary: lhsT = wb [128ki, 2ko, 128o] reused
   by 2 moving-x matmuls (rhs = xb [128ki, 2ko, 512t], out psum [128t?o, 512]).
   Probes showed LDWEIGHTS switch cost and ifmap interleaving are ~free; the
   ~280ns/MM (vs 241 ideal) is intrinsic (P0 clock under sustained load).
 - The timing loop body holds EIGHT unrolled copies with explicit
   tc.stage_boundary() every 2 copies (staggered_reset=True): stage ==
   2 iterations, so the adjacent-stage rule gives copy N+1's prep a full
   matmul-span of lookahead. (The auto equal split cuts mid-MM-stream and
   couples PE to the DMA stream; full-barrier mode kills cross-body overlap.)
 - ENGINE-PHASE PURITY (the critical fix): every engine's FIFO queue holds
   only same-phase work, so copy N's late-paced work never blocks copy N+1's
   prep: SP = loads only; ACT = early signs only (x0,x1,w0-w4, native Sign);
   DVE = late signs (w5-w7, two-op min/max form, exact for |v| >= 2^-126)
   then PSUM evictions (tensor_scalar_add with per-partition bias AP);
   GPS/SWDGE = store + bias DMA issues only (GPS tensor compute is ~45us/op —
   never use it for elementwise).
 - All operand pools bufs=2 (copies alternate slots) => cross-iteration
   software pipelining; staging pools sized so WAR slot releases land early.
"""

import numpy as np

N_CORES = 8
TOKENS = 8192
D_IN = 2048
D_OUT = 2048
T_CORE = TOKENS // N_CORES  # 1024 tokens per core

P = 128
KO = D_IN // P          # 16 contraction chunks of 128
KP = KO // 2            # 8 DoubleRow K-pairs (256 per matmul)
NB = 512                # matmul moving free dim / PSUM bank (fp32)
TH = T_CORE // NB       # 2 token halves of 512
OC = D_OUT // P         # 16 out-feature tiles of 128
WCH = 8                 # W prep chunks (256 o-rows each)
WCO = D_OUT // WCH      # 256 o per W chunk

_CACHE = {}
LAST_RESULT = None

# "wstat": DoubleRow, W stationary, wb [P, KO, WCO]
# "wswi":  DoubleRowSwInterleave — host pre-interleaves W so LDWEIGHTS reads
#          contiguously; wb [P, KP, 2, 2P]
MM_STRUCT = "wstat"


def _build_bass(loop_n=1, phase="all", mm_struct=None, sign_gps=False,
                loop_mode="staggered16"):
    if mm_struct is None:
        mm_struct = MM_STRUCT
    import concourse.mybir as mybir
    import concourse.tile as tile
    from concourse import bacc
    from concourse.bass import ts

    nc = bacc.Bacc(
        "TRN2",
        target_bir_lowering=False,
        debug=False,
        enable_asserts=False,
    )

    f32 = mybir.dt.float32
    bf16 = mybir.dt.bfloat16
    fp16 = mybir.dt.float16
    fp8 = mybir.dt.float8e4

    # xp/Wp are host-permuted to the contraction-major SBUF layout
    # [chunk][ki, ko, t|o] (bit-exact bf16 values, pure layout staging), so
    # each device load is one fully-contiguous-per-partition plain DMA — no
    # xbar DMA-transpose, max descriptor efficiency.
    x_d = nc.dram_tensor("xp", [TH, P, KO, NB], bf16, kind="ExternalInput")
    if mm_struct == "wswi":
        w_d = nc.dram_tensor("Wp", [WCH, P, KP, 2, 2 * P], bf16,
                             kind="ExternalInput")
    else:
        w_d = nc.dram_tensor("Wp", [WCH, P, KO, WCO], bf16, kind="ExternalInput")
    b_d = nc.dram_tensor("bt", [P, OC], f32, kind="ExternalInput")
    y_d = nc.dram_tensor("yT", [D_OUT, T_CORE], fp16, kind="ExternalOutput")

    x_ap = x_d.ap()
    w_ap = w_d.ap()
    b_ap = b_d.ap()
    y_ap = y_d.ap()

    if loop_n <= 1:
        unroll = 1
    elif loop_mode == "staggered16":
        unroll = 16
    elif loop_mode == "staggered8":
        unroll = 8
    else:
        unroll = 4

    with tile.TileContext(nc) as tc:
        with (
            tc.tile_pool(name="ops", bufs=2) as ops,
            tc.tile_pool(name="xstage", bufs=2) as xstage,
            tc.tile_pool(name="wstage", bufs=6) as wstage,
            tc.tile_pool(name="dvetmp", bufs=1) as dvetmp,
            tc.tile_pool(name="outp", bufs=3) as out_pool,
            tc.tile_pool(name="psum", bufs=8, space="PSUM") as psum_pool,
        ):
            ev_i = 0

            def body_one(u):
                nonlocal ev_i
                # --- operand tiles for this (unrolled) iteration ---
                if mm_struct == "xswi":
                    # moving-operand pairs contiguous: [ki, kp, t, b]
                    xb = [ops.tile([P, KP, NB, 2], fp8, name=f"xb{h}")
                          for h in range(TH)]
                else:
                    xb = [ops.tile([P, KO, NB], fp8, name=f"xb{h}") for h in range(TH)]
                if mm_struct in ("wstat", "wsame", "xswi"):
                    wb = [ops.tile([P, KO, WCO], fp8, name=f"wb{c}") for c in range(WCH)]
                elif mm_struct == "wswi":
                    # SwInterleave storage: per (kp, lo) a contiguous 256-elem
                    # block [A_col127, B_col127, A_col126, ..., B_col0]
                    wb = [ops.tile([P, KP, 2, 2 * P], fp8, name=f"wb{c}")
                          for c in range(WCH)]
                else:
                    wb = [ops.tile([P, KO, NB], fp8, name=f"wb{c}") for c in range(4)]
                bias = ops.tile([P, OC], f32, name="bias")

                if phase == "mm":
                    # timing-only build: tiny slice writes allocate the tiles
                    # (full contents are garbage; numerics unused)
                    for t_ in xb + wb:
                        if (mm_struct == "wswi" and t_ in wb) or (
                            mm_struct == "xswi" and t_ in xb
                        ):
                            nc.gpsimd.memset(t_[:, 0, 0, 0:1], 1.0)
                        else:
                            nc.gpsimd.memset(t_[:, 0, 0:1], 1.0)
                    nc.gpsimd.memset(bias[:, 0:1], 0.0)
                else:
                    # --- prep: DMA-transpose from DRAM (bf16) + sign -> fp8 ---
                    nc.gpsimd.dma_start(bias[:], b_ap[:, :])

                    def sign_act(dst, src):
                        nc.scalar.sign(dst, src)

                    def sign_2op(eng, dst, src, shape):
                        # exact sign for all |v| >= 2^-126 (incl. v == 0)
                        tmp = dvetmp.tile(shape, bf16, name="dvetmp")
                        eng.tensor_scalar(
                            tmp[:], src, 2.0 ** 126, 1.0,
                            mybir.AluOpType.mult, mybir.AluOpType.min,
                        )
                        eng.tensor_scalar_max(dst, tmp[:], -1.0)

                    def prep_x(h):
                        st = xstage.tile([P, KO, NB], bf16, name="xst")
                        nc.sync.dma_start(st[:], x_ap[h])
                        if phase == "dma":
                            nc.vector.tensor_copy(xb[h][:, 0, 0:1], st[:, 0, 0:1])
                            return
                        sign_act(xb[h][:], st[:])

                    def prep_w(c):
                        wst_shape = (
                            [P, KP, 2, 2 * P] if mm_struct == "wswi"
                            else [P, KO, WCO]
                        )
                        st = wstage.tile(wst_shape, bf16, name="wst")
                        nc.sync.dma_start(st[:], w_ap[c])
                        if phase == "dma":
                            dst = (wb[c][:, 0, 0, 0:1] if mm_struct == "wswi"
                                   else wb[c][:, 0, 0:1])
                            nc.vector.tensor_copy(dst, st[:, 0, 0:1]
                                                  if mm_struct != "wswi"
                                                  else st[:, 0, 0, 0:1])
                            return
                        # DVE gets the LATE-consumed chunks (w5..w7): its queue
                        # drains copy u's evictions first, so u+1's DVE signs
                        # land mid-mm — early chunks must come from ACT, whose
                        # queue holds only signs and drains well before u ends.
                        if c >= 5:
                            sign_2op(nc.vector, wb[c][:], st[:], wst_shape)
                        else:
                            sign_act(wb[c][:], st[:])

                    prep_x(0)
                    prep_w(0)
                    prep_x(1)
                    for c in range(1, WCH):
                        prep_w(c)

                if phase in ("prep", "dma"):
                    # tiny consumers so prep work can't be dead-code'd away
                    o_sb = out_pool.tile([P, NB], fp16, tag="osb", name="o_sb")
                    for i, t_ in enumerate(xb + wb):
                        src = (t_[:, 0, 0, 0:1]
                               if mm_struct == "wswi" and t_ not in xb
                               else t_[:, 0, 0:1])
                        nc.vector.tensor_copy(o_sb[:, i : i + 1], src)
                    nc.scalar.dma_start(y_ap[ts(0, P), ts(0, NB)], o_sb[:])
                    return

                if mm_struct in ("wstat", "wsame", "wswi", "xswi"):
                    # th INNER: each stationary W tile (LDWEIGHTS) is reused by
                    # the 2 moving-x matmuls — measured ~5us/iter cheaper than
                    # reloading per MM (th-outer). In steady state prep(u+1)
                    # completes during mm(u), so x-half gating doesn't matter.
                    for oc in range(OC):
                        c, lo = divmod(oc, 2)
                        if mm_struct == "wsame":
                            c, lo = 0, 0  # fixed stationary: LDW-elision probe
                        psums = [
                            psum_pool.tile([P, NB], f32, tag="psum", name="psum")
                            for _ in range(TH)
                        ]
                        for kp in range(KP):
                            kp_ = 0 if mm_struct == "wsame" else kp
                            if mm_struct == "wswi":
                                lhsT = wb[c][:, kp_, lo, :]
                                pm = mybir.MatmulPerfMode.DoubleRowSwInterleave
                            else:
                                lhsT = wb[c][:, 2 * kp_ : 2 * kp_ + 2, ts(lo, P)]
                                pm = mybir.MatmulPerfMode.DoubleRow
                            for th in range(TH):
                                if mm_struct == "xswi":
                                    rhs = xb[th][:, kp, :, :].transpose([0, 2, 1])
                                else:
                                    rhs = xb[th][:, 2 * kp : 2 * kp + 2, :]
                                nc.tensor.matmul(
                                    psums[th][:],
                                    lhsT=lhsT,
                                    rhs=rhs,
                                    perf_mode=pm,
                                    start=(kp == 0),
                                    stop=(kp == KP - 1),
                                )
                        for th in range(TH):
                            o_sb = out_pool.tile([P, NB], fp16, tag="osb", name="o_sb")
                            # evictions on DVE (ACT Identity would force
                            # activation-table switches against the Sign ops)
                            nc.vector.tensor_scalar_add(
                                o_sb[:], psums[th][:], bias[:, oc : oc + 1]
                            )
                            ev_i += 1
                            # stores issue via GPS/SWDGE: the store issues are
                            # paced by the mm span, and any engine that also
                            # did sign work would FIFO-block the next copy's
                            # signs behind them. GPS does only stores + bias.
                            nc.gpsimd.dma_start(y_ap[ts(oc, P), ts(th, NB)], o_sb[:])
                else:
                    # xstat: v1-style — stationary x token-tile, moving W bank
                    # [128, 2, 512]; timing-only build (phase="mm").
                    assert phase == "mm"
                    for ob in range(4):
                        for tt in range(8):
                            th, tl = divmod(tt, 4)
                            psum = psum_pool.tile([P, NB], f32, tag="psum", name="psum")
                            for kp in range(KP):
                                nc.tensor.matmul(
                                    psum[:],
                                    lhsT=xb[th][:, 2 * kp : 2 * kp + 2, ts(tl, P)],
                                    rhs=wb[ob][:, 2 * kp : 2 * kp + 2, :],
                                    perf_mode=mybir.MatmulPerfMode.DoubleRow,
                                    start=(kp == 0),
                                    stop=(kp == KP - 1),
                                )
                            o_sb = out_pool.tile([P, NB], fp16, tag="osb", name="o_sb")
                            nc.vector.tensor_scalar_add(
                                o_sb[:], psum[:], bias[:, 0:1]
                            )
                            # timing-only: yT is [2048, 1024]; write any
                            # distinct in-range region per (ob, tt)
                            nc.scalar.dma_start(
                                y_ap[ts(2 * ob + (tt % 2), P), ts(tt // 4, NB)],
                                o_sb[:],
                            )

            staggered = loop_mode in ("staggered4", "staggered8", "staggered16")
            copies_per_stage = unroll // 4 if staggered else unroll

            def body():
                for u in range(unroll):
                    if staggered and u > 0 and u % copies_per_stage == 0:
                        # per-copy-group stage boundaries: stage == 1-2 logical
                        # iterations, so the adjacent-stage rule allows 1-3
                        # copies of prep lookahead (the auto equal split cuts
                        # mid-MM-stream and couples PE to the DMA/sign stream)
                        tc.stage_boundary()
                    body_one(u)

            if loop_n > 1:
                assert loop_n % unroll == 0
                with tc.For_i(
                    0,
                    loop_n // unroll,
                    1,
                    hint_engines=(mybir.EngineType.PE,),
                    staggered_reset=staggered,
                ):
                    body()
            else:
                body()

    nc.compile()
    return nc


def _get_nc():
    if "nc" not in _CACHE:
        _CACHE["nc"] = _build_bass()
    return _CACHE["nc"]


def _host_inputs(inputs):
    import ml_dtypes

    x = np.asarray(inputs["x"], dtype=np.float32)
    W = np.asarray(inputs["W"], dtype=np.float32)
    b = np.ascontiguousarray(np.asarray(inputs["b"], dtype=np.float32))

    # bf16 staging: sign-preserving (bf16 keeps fp32's exponent range);
    # layouts permuted to contraction-major [ki, ko, t|o]
    x16 = x.astype(ml_dtypes.bfloat16)
    W16 = W.astype(ml_dtypes.bfloat16)
    xp = [
        np.ascontiguousarray(
            x16[c * T_CORE : (c + 1) * T_CORE]
            .reshape(TH, NB, KO, P)
            .transpose(0, 3, 2, 1)
        )
        for c in range(N_CORES)
    ]
    if MM_STRUCT == "wswi":
        # SwInterleave weight staging: per (c, ki, kp, lo) a 256-elem block
        # [A_col127, B_col127, A_col126, ..., B_col0]; A/B = ko-even/odd,
        # columns descending (HW deinterleaves then reverses)
        arr = W16.reshape(WCH, 2, P, KP, 2, P)  # [c, lo, m, kp, b, ki]
        Wp = np.ascontiguousarray(
            arr[:, :, ::-1]
            .transpose(0, 5, 3, 1, 2, 4)
            .reshape(WCH, P, KP, 2, 2 * P)
        )
    else:
        Wp = np.ascontiguousarray(
            W16.reshape(WCH, WCO, KO, P).transpose(0, 3, 2, 1)
        )
    # bias transposed to per-partition layout: bt[p, c] = b[c*128 + p]
    bt = np.ascontiguousarray(b.reshape(OC, P).T)
    return xp, Wp, bt


def kernel(**inputs):
    global LAST_RESULT

    from concourse.bass_utils import run_bass_kernel_spmd

    xp, Wp, bt = _host_inputs(inputs)

    nc = _get_nc()
    in_maps = [
        {"xp": xp[c], "Wp": Wp, "bt": bt}
        for c in range(N_CORES)
    ]
    res = run_bass_kernel_spmd(nc, in_maps, core_ids=list(range(N_CORES)))
    LAST_RESULT = res
    # un-transpose per-core yT [2048, 1024] -> y [1024, 2048]; widen to f32
    y = np.concatenate(
        [np.ascontiguousarray(r["yT"].T) for r in res.results], axis=0
    )
    return y.astype(np.float32)
